# revision 38
# baseline (speedup 1.0000x reference)
"""CNLP (common-neighbor link prediction) kernel for Trainium2, 8 NeuronCores.

Reference computation (per query edge e = (i, j)):
    cn  = adj[i] * adj[j]                      # common-neighbor indicator [N]
    xcn = cn @ x                               # sum of common-neighbor feats
    xij = relu(x[i]*x[j] @ Wa.T + ba) @ Wb.T + bb
    hcn = (relu->relu->lin) 3-layer MLP on xcn
    out = (hcn * beta + xij) @ Wl.T + bl       # [E, 1]

Sharding: edges (E=8192) split 8 x 1024 across cores; adj/x/weights replicated.

Device strategy per core (1024 edges in 2 blocks of 512):
  - adj is binary -> uploaded as fp8e4 (exact), padded N->10240.
  - Per (block, n-quarter): ONE gpsimd dma_gather(transpose=True) pulls the
    adjacency rows for idx list [e0 || e1] (1024 idxs x 2560 B) into
    [128, 10, 2, 1024] fp8; DVE ANDs the two halves in 16-bit 2x mode -> cn.
  - Big matmul is FLIPPED vs the e-major formulation: stationary = fp8 x
    table (host-permuted to match the 16-bit gather interleave), moving =
    cn slices; PSUM accumulates xcn^T feature-major [128f, 512e] directly
    (no PE transposes).  DOUBLE_ROW uses fp8 DoubleRow perf mode (2 k-tiles
    = adjacent word-chunks) for 2x PE throughput.
  - MLPs run feature-major with fp32 weights/activations read as float32r
    (fp22) -> 1 cycle/row instead of 4.  xij path gathers bf16 x rows.
"""

import numpy as np
import ml_dtypes

import concourse.bacc as bacc
import concourse.tile as tile
import concourse.mybir as mybir
from concourse.bass_utils import run_bass_kernel_spmd

BF16 = mybir.dt.bfloat16
FP32 = mybir.dt.float32
FP32R = mybir.dt.float32r
FP8 = mybir.dt.float8e4
I16 = mybir.dt.int16
AF = mybir.ActivationFunctionType
BF16_NP = ml_dtypes.bfloat16
FP8_NP = ml_dtypes.float8_e4m3

N_CORES = 8
N, E, D, H = 10000, 8192, 256, 256
NPAD = 10240                      # n padded to a multiple of 256
EC = E // N_CORES                 # 1024 edges per core
EB = 512                          # edges per block
NB = EC // EB                     # 2 blocks
# adjacency rows extended with the bf16 x row: [adj fp8 10240B | x bf16 512B];
# gathered in two uneven splits (both %256, even DoubleRow pair counts)
ROWB = NPAD + 2 * D               # 10752 bytes per extended row
QOFF = (0, 5632)                  # byte offset of each split
QELEM = (5632, 5120)              # gather elem bytes (split 1 carries the x row)
QC2 = (22, 18)                    # adjacency word-chunks (256 nodes) per split
GBASE = (0, 22)                   # global word-chunk base per split
NGC = 40                          # total word-chunks (10240 nodes)

DOUBLE_ROW = True


def build_program():
    nc = bacc.Bacc("TRN2", target_bir_lowering=False, debug=False,
                   enable_asserts=False, num_devices=N_CORES)

    adjx = nc.dram_tensor("adjx", [N, ROWB], FP8, kind="ExternalInput")
    x8t_d = nc.dram_tensor("x8t", [128, NGC * 2 * 2 * 128], FP8,
                           kind="ExternalInput")
    # gather idx (raw node ids), 512-idx wraps ordered (b, src); shared by
    # the adjacency gathers and the (unpermuted) xh gathers
    idxg_d = nc.dram_tensor("idxg", [128, NB * 2 * EB // 16], I16,
                            kind="ExternalInput")
    wts_d = {nm: nc.dram_tensor(nm, [D, H], FP32, kind="ExternalInput")
             for nm in ("wat", "wbt", "w1t", "w2t", "w3t")}
    wlt_d = nc.dram_tensor("wlt", [H, 1], FP32, kind="ExternalInput")
    bias_d = {nm: nc.dram_tensor(nm, [2, 128, 1], FP32, kind="ExternalInput")
              for nm in ("ba", "bb", "b1", "b2", "b3")}
    bl_d = nc.dram_tensor("bl", [1, 1], FP32, kind="ExternalInput")
    beta_d = nc.dram_tensor("beta", [128, 1], FP32, kind="ExternalInput")
    out_d = nc.dram_tensor("out", [1, EC], FP32, kind="ExternalOutput")

    with tile.TileContext(nc) as tc:
        with (
            tc.tile_pool(name="const", bufs=1) as constp,
            tc.tile_pool(name="gath", bufs=4) as gathp,
            tc.tile_pool(name="acts", bufs=6) as actp,
            tc.tile_pool(name="px", bufs=4, space="PSUM") as pxp,
            tc.tile_pool(name="pm", bufs=2, space="PSUM") as pmp,
            tc.tile_pool(name="po", bufs=2, space="PSUM") as pop,
        ):
            # ---- persistent loads -------------------------------------
            # idx tiles FIRST (gathers wait on them; HWDGE is FIFO)
            idxg_sb = constp.tile([128, NB * 2 * EB // 16], I16)
            nc.sync.dma_start(idxg_sb[:], idxg_d[:])

            # fp8 x table, split in 4 loads so early matmuls start sooner
            x8t_sb = constp.tile([128, NGC, 2, 2, 128], FP8)
            qsz = (NGC // 4) * 2 * 2 * 128
            for q in range(4):
                nc.sync.dma_start(
                    x8t_sb[:, q * (NGC // 4):(q + 1) * (NGC // 4), :, :, :]
                    .rearrange("p c t f g -> p (c t f g)"),
                    x8t_d[:, q * qsz:(q + 1) * qsz])

            # weights: DMA fp32 staging -> scalar-copy to fp32r (the BIR
            # verifier requires fp32r matmul operands to be fp32r-rounded)
            w_sb = {}
            for nm, d in wts_d.items():
                stg = constp.tile([128, 2, H], FP32, tag="wstg", bufs=2,
                                  name=f"ws_{nm}")
                nc.sync.dma_start(stg[:], d[:].rearrange("(k p) h -> p k h", p=128))
                t = constp.tile([128, 2, H], FP32R, tag=f"w_{nm}")
                nc.scalar.activation(t[:], stg[:], AF.Copy)
                w_sb[nm] = t
            wlt_stg = constp.tile([128, 2, 1], FP32)
            nc.sync.dma_start(wlt_stg[:], wlt_d[:].rearrange("(k p) o -> p k o", p=128))
            wlt_sb = constp.tile([128, 2, 1], FP32R)
            nc.scalar.activation(wlt_sb[:], wlt_stg[:], AF.Copy)
            b_sb = {}
            for nm, d in bias_d.items():
                t = constp.tile([128, 2, 1], FP32, tag=f"b_{nm}")
                nc.sync.dma_start(t[:], d[:].rearrange("t p o -> p t o"))
                b_sb[nm] = t
            bl_sb = constp.tile([1, 1], FP32)
            nc.sync.dma_start(bl_sb[:], bl_d[:])
            beta_sb = constp.tile([128, 1], FP32)
            nc.sync.dma_start(beta_sb[:], beta_d[:])

            out_sb = constp.tile([1, EC], FP32)

            # MLP layer, feature-major fp32r (fp22 reads, 1 cyc/row), 512 edges
            def lin_h(src, wname, bname, relu, dst):
                w, bias = w_sb[wname], b_sb[bname]
                for t in range(2):
                    pm = pmp.tile([128, EB], FP32, tag="pm")
                    for k in range(2):
                        nc.tensor.matmul(
                            pm[:], w[:, k, t * 128:(t + 1) * 128],
                            src[:, k, :], start=(k == 0), stop=(k == 1))
                    dsl = dst[:, t, :]
                    if t % 2 == 0:
                        nc.scalar.activation(
                            dsl, pm[:], AF.Relu if relu else AF.Identity,
                            bias=bias[:, t, :])
                    elif relu:
                        nc.vector.tensor_scalar(
                            dsl, pm[:], bias[:, t, :], 0.0,
                            mybir.AluOpType.add, mybir.AluOpType.max)
                    else:
                        nc.vector.tensor_scalar_add(dsl, pm[:], bias[:, t, :])
                return dst

            def mlp_block(b, xcn_sb, xiT, xjT):
                pT = actp.tile([128, 2, EB], FP32R, tag="act")
                nc.vector.tensor_mul(pT[:], xiT, xjT)
                u = lin_h(pT, "wat", "ba", True,
                          actp.tile([128, 2, EB], FP32R, tag="act", name=f"u{b}"))
                xijT = lin_h(u, "wbt", "bb", False,
                             actp.tile([128, 2, EB], FP32R, tag="act",
                                       name=f"xij{b}"))
                h = xcn_sb
                for li, (wn, bn, rl) in enumerate((
                        ("w1t", "b1", True), ("w2t", "b2", True),
                        ("w3t", "b3", False))):
                    h = lin_h(h, wn, bn, rl,
                              actp.tile([128, 2, EB], FP32R, tag="act",
                                        name=f"h{b}_{li}"))
                nc.vector.tensor_scalar_mul(h[:], h[:], beta_sb[:])
                nc.vector.tensor_add(h[:], h[:], xijT[:])
                po = pop.tile([1, EB], FP32, tag="po")
                for k in range(2):
                    nc.tensor.matmul(po[:], wlt_sb[:, k, :], h[:, k, :],
                                     start=(k == 0), stop=(k == 1))
                nc.scalar.activation(out_sb[:, b * EB:(b + 1) * EB],
                                     po[:], AF.Identity, bias=bl_sb[:])

            # ---- main loop: gather -> AND -> xcn^T matmul ------------
            for b in range(NB):
                px = [pxp.tile([128, EB], FP32, tag="px", name=f"px{b}_{fh}")
                      for fh in range(2)]
                xv = None
                for q in range(2):
                    nch = QELEM[q] // 128          # byte-chunks in tile
                    ac2 = QC2[q]                   # adjacency word-chunks
                    ab = []
                    for s in range(2):
                        gsl = slice((2 * b + s) * EB // 16,
                                    (2 * b + s + 1) * EB // 16)
                        t = gathp.tile([128, nch, EB], FP8, tag=f"g{q}",
                                       bufs=3, name=f"a{b}{q}{s}")
                        nc.gpsimd.dma_gather(
                            t[:], adjx[:, QOFF[q]:QOFF[q] + QELEM[q]],
                            idxg_sb[:, gsl], EB, EB,
                            elem_size=QELEM[q], elem_step=ROWB, transpose=True)
                        ab.append(t)
                    if q == 1:
                        # bf16 x rows ride in the last 4 byte-chunks:
                        # feature (fh*128+p), edge i at [p, 2*ac2+2*fh+i//256,
                        # i%256] of the bf16 view -> [128, 2, 512]
                        xv = [t[:].bitcast(BF16)[:, 2 * ac2:2 * ac2 + 4, :]
                              .rearrange("p (f s) w -> p f (s w)", f=2)
                              for t in ab]
                    # cn = a0 AND a1 (binary fp8: bitwise AND == product),
                    # in place into a0, contiguous u16 for DVE 2x mode;
                    # split for AND->matmul overlap, adjacency chunks only
                    v0 = ab[0][:].bitcast(I16)
                    v1 = ab[1][:].bitcast(I16)
                    for hh in range(2):
                        csl = slice(hh * ac2, (hh + 1) * ac2)
                        nc.vector.tensor_tensor(
                            v0[:, csl, :], v0[:, csl, :], v1[:, csl, :],
                            mybir.AluOpType.bitwise_and)
                    # cn byte at (c2, par, i): flat = c2*1024 + 2i + par
                    va = (ab[0][:, 0:2 * ac2, :]
                          .rearrange("p cb j -> p (cb j)")
                          .rearrange("p (c i two) -> p c two i",
                                     c=ac2, two=2))
                    for w2 in range(ac2 // 2):
                        gc = GBASE[q] + 2 * w2
                        for par in range(2):
                            for fh in range(2):
                                if DOUBLE_ROW:
                                    nc.tensor.matmul(
                                        px[fh][:],
                                        x8t_sb[:, gc:gc + 2, par, fh, :],
                                        va[:, 2 * w2:2 * w2 + 2, par, :],
                                        start=(q == 0 and w2 == 0 and par == 0),
                                        stop=(q == 1 and w2 == ac2 // 2 - 1
                                              and par == 1),
                                        perf_mode=mybir.MatmulPerfMode.DoubleRow)
                                else:
                                    for t2 in range(2):
                                        nc.tensor.matmul(
                                            px[fh][:],
                                            x8t_sb[:, gc + t2, par, fh, :],
                                            va[:, 2 * w2 + t2, par, :],
                                            start=(q == 0 and w2 == 0
                                                   and par == 0 and t2 == 0),
                                            stop=(q == 1 and w2 == ac2 // 2 - 1
                                                  and par == 1 and t2 == 1))
                xcn_sb = actp.tile([128, 2, EB], FP32R, tag="act",
                                   name=f"xcn{b}")
                for fh in range(2):
                    nc.scalar.activation(xcn_sb[:, fh, :], px[fh][:], AF.Copy)
                mlp_block(b, xcn_sb, xv[0], xv[1])

            nc.sync.dma_start(out_d[:], out_sb[:])

    nc.compile()
    return nc


def _wrap_idx(ids, num):
    """Pack indices for dma_gather: [128, num//16] int16, idx i at
    [i % 16, i // 16], replicated over the 8 groups of 16 partitions."""
    a = np.asarray(ids).astype(np.int16)
    w = a.reshape(num // 16, 16).T.copy()
    return np.ascontiguousarray(np.tile(w, (8, 1)))


def prepare_inputs(x, adj, edge, W1, b1, W2, b2, W3, b3, Wa, ba, Wb, bb,
                   Wl, bl, beta):
    x = np.asarray(x, np.float32)
    adj = np.asarray(adj, np.float32)
    edge = np.asarray(edge)

    # extended rows: [adj fp8 (10240B) | x bf16 (512B)]
    adjx8 = np.zeros((N, ROWB), np.uint8)
    adjx8[:, :N] = adj.astype(FP8_NP).view(np.uint8)
    adjx8[:, NPAD:] = np.ascontiguousarray(
        x.astype(BF16_NP)).view(np.uint8).reshape(N, 2 * D)
    adjx = adjx8.view(FP8_NP)

    # fp8 x table, permuted to the gather interleave:
    # x8t[p, gc, par, fh, f] = x8[gc*256 + 2p + par, fh*128 + f]
    x8 = np.zeros((NPAD, D), FP8_NP)
    x8[:N] = x.astype(FP8_NP)
    x8t = np.ascontiguousarray(
        x8.reshape(NGC, 128, 2, 2, 128)
        .transpose(1, 0, 2, 3, 4).reshape(128, -1))

    common = dict(
        adjx=adjx, x8t=x8t,
        wat=np.ascontiguousarray(np.asarray(Wa, np.float32).T),
        wbt=np.ascontiguousarray(np.asarray(Wb, np.float32).T),
        w1t=np.ascontiguousarray(np.asarray(W1, np.float32).T),
        w2t=np.ascontiguousarray(np.asarray(W2, np.float32).T),
        w3t=np.ascontiguousarray(np.asarray(W3, np.float32).T),
        wlt=np.ascontiguousarray(np.asarray(Wl, np.float32).T),
        ba=np.asarray(ba, np.float32).reshape(2, 128, 1),
        bb=np.asarray(bb, np.float32).reshape(2, 128, 1),
        b1=np.asarray(b1, np.float32).reshape(2, 128, 1),
        b2=np.asarray(b2, np.float32).reshape(2, 128, 1),
        b3=np.asarray(b3, np.float32).reshape(2, 128, 1),
        bl=np.asarray(bl, np.float32).reshape(1, 1),
        beta=np.full((128, 1), np.asarray(beta, np.float32).reshape(-1)[0],
                     np.float32),
    )
    in_maps = []
    for c in range(N_CORES):
        m = dict(common)
        gi = []
        for b in range(NB):
            sl = slice(c * EC + b * EB, c * EC + (b + 1) * EB)
            for s in range(2):
                gi.append(_wrap_idx(edge[sl, s], EB))
        m["idxg"] = np.ascontiguousarray(np.hstack(gi))
        in_maps.append(m)
    return in_maps


_CACHE = {}


def _get_program():
    if "nc" not in _CACHE:
        _CACHE["nc"] = build_program()
    return _CACHE["nc"]


def run(in_maps, **kw):
    nc = _get_program()
    return run_bass_kernel_spmd(nc, in_maps, list(range(N_CORES)), **kw)


def kernel(**inputs):
    in_maps = prepare_inputs(**inputs)
    res = run(in_maps)
    out = np.concatenate([res.results[c]["out"][0] for c in range(N_CORES)])
    return out.reshape(E, 1).astype(np.float32)


# revision 42
# speedup vs baseline: 1.3449x; 1.3449x over previous
"""CNLP (common-neighbor link prediction) kernel for Trainium2, 8 NeuronCores.

Reference computation (per query edge e = (i, j)):
    cn  = adj[i] * adj[j]                      # common-neighbor indicator [N]
    xcn = cn @ x                               # sum of common-neighbor feats
    xij = relu(x[i]*x[j] @ Wa.T + ba) @ Wb.T + bb
    hcn = (relu->relu->lin) 3-layer MLP on xcn
    out = (hcn * beta + xij) @ Wl.T + bl       # [E, 1]

Sharding: edges (E=8192) split 8 x 1024 across cores; adj/x/weights replicated.

Device strategy per core (1024 edges in 2 blocks of 512):
  - adj is binary -> uploaded as fp8e4 (exact), padded N->10240.
  - Per (block, n-quarter): ONE gpsimd dma_gather(transpose=True) pulls the
    adjacency rows for idx list [e0 || e1] (1024 idxs x 2560 B) into
    [128, 10, 2, 1024] fp8; DVE ANDs the two halves in 16-bit 2x mode -> cn.
  - Big matmul is FLIPPED vs the e-major formulation: stationary = fp8 x
    table (host-permuted to match the 16-bit gather interleave), moving =
    cn slices; PSUM accumulates xcn^T feature-major [128f, 512e] directly
    (no PE transposes).  DOUBLE_ROW uses fp8 DoubleRow perf mode (2 k-tiles
    = adjacent word-chunks) for 2x PE throughput.
  - MLPs run feature-major with fp32 weights/activations read as float32r
    (fp22) -> 1 cycle/row instead of 4.  xij path gathers bf16 x rows.
"""

import numpy as np
import ml_dtypes

import concourse.bacc as bacc
import concourse.tile as tile
import concourse.mybir as mybir
from concourse.bass_utils import run_bass_kernel_spmd

BF16 = mybir.dt.bfloat16
FP32 = mybir.dt.float32
FP32R = mybir.dt.float32r
FP8 = mybir.dt.float8e4
I16 = mybir.dt.int16
AF = mybir.ActivationFunctionType
BF16_NP = ml_dtypes.bfloat16
FP8_NP = ml_dtypes.float8_e4m3

N_CORES = 8
N, E, D, H = 10000, 8192, 256, 256
NPAD = 10240                      # n padded to a multiple of 256
EC = E // N_CORES                 # 1024 edges per core
EB = 512                          # edges per block
NB = EC // EB                     # 2 blocks
# adjacency rows extended with the bf16 x row: [adj fp8 10240B | x bf16 512B];
# gathered in four splits (all %256), the last carrying the x row
ROWB = NPAD + 2 * D               # 10752 bytes per extended row
NSPLIT = 4
QOFF = (0, 2560, 5120, 7680)      # byte offset of each split
QELEM = (2560, 2560, 2560, 3072)  # gather elem bytes
QC2 = (10, 10, 10, 10)            # adjacency word-chunks (256 nodes) per split
GBASE = (0, 10, 20, 30)           # global word-chunk base per split
NGC = 40                          # total word-chunks (10240 nodes)

DOUBLE_ROW = True


def build_program():
    nc = bacc.Bacc("TRN2", target_bir_lowering=False, debug=False,
                   enable_asserts=False, num_devices=N_CORES)

    adjx = nc.dram_tensor("adjx", [N, ROWB], FP8, kind="ExternalInput")
    x8t_d = nc.dram_tensor("x8t", [128, NGC * 2 * 2 * 128], FP8,
                           kind="ExternalInput")
    # gather idx (raw node ids), 512-idx wraps ordered (b, src); shared by
    # the adjacency gathers and the (unpermuted) xh gathers
    idxg_d = nc.dram_tensor("idxg", [128, NB * 2 * EB // 16], I16,
                            kind="ExternalInput")
    wts_d = {nm: nc.dram_tensor(nm, [D, H], FP32, kind="ExternalInput")
             for nm in ("wat", "wbt", "w1t", "w2t", "w3t")}
    wlt_d = nc.dram_tensor("wlt", [H, 1], FP32, kind="ExternalInput")
    bias_d = {nm: nc.dram_tensor(nm, [2, 128, 1], FP32, kind="ExternalInput")
              for nm in ("ba", "bb", "b1", "b2", "b3")}
    bl_d = nc.dram_tensor("bl", [1, 1], FP32, kind="ExternalInput")
    beta_d = nc.dram_tensor("beta", [128, 1], FP32, kind="ExternalInput")
    out_d = nc.dram_tensor("out", [1, EC], FP32, kind="ExternalOutput")

    with tile.TileContext(nc) as tc:
        with (
            tc.tile_pool(name="const", bufs=1) as constp,
            tc.tile_pool(name="gath", bufs=4) as gathp,
            tc.tile_pool(name="acts", bufs=6) as actp,
            tc.tile_pool(name="px", bufs=4, space="PSUM") as pxp,
            tc.tile_pool(name="pm", bufs=2, space="PSUM") as pmp,
            tc.tile_pool(name="po", bufs=2, space="PSUM") as pop,
        ):
            # ---- persistent loads -------------------------------------
            # idx tiles FIRST (gathers wait on them; HWDGE is FIFO)
            idxg_sb = constp.tile([128, NB * 2 * EB // 16], I16)
            nc.sync.dma_start(idxg_sb[:], idxg_d[:])

            # fp8 x table, split in 4 loads so early matmuls start sooner
            x8t_sb = constp.tile([128, NGC, 2, 2, 128], FP8)
            qsz = (NGC // 4) * 2 * 2 * 128
            for q in range(4):
                nc.sync.dma_start(
                    x8t_sb[:, q * (NGC // 4):(q + 1) * (NGC // 4), :, :, :]
                    .rearrange("p c t f g -> p (c t f g)"),
                    x8t_d[:, q * qsz:(q + 1) * qsz])

            # weights: DMA fp32 staging -> scalar-copy to fp32r (the BIR
            # verifier requires fp32r matmul operands to be fp32r-rounded)
            w_sb = {}
            for nm, d in wts_d.items():
                stg = constp.tile([128, 2, H], FP32, tag="wstg", bufs=2,
                                  name=f"ws_{nm}")
                nc.sync.dma_start(stg[:], d[:].rearrange("(k p) h -> p k h", p=128))
                t = constp.tile([128, 2, H], FP32R, tag=f"w_{nm}")
                nc.scalar.activation(t[:], stg[:], AF.Copy)
                w_sb[nm] = t
            wlt_stg = constp.tile([128, 2, 1], FP32)
            nc.sync.dma_start(wlt_stg[:], wlt_d[:].rearrange("(k p) o -> p k o", p=128))
            wlt_sb = constp.tile([128, 2, 1], FP32R)
            nc.scalar.activation(wlt_sb[:], wlt_stg[:], AF.Copy)
            b_sb = {}
            for nm, d in bias_d.items():
                t = constp.tile([128, 2, 1], FP32, tag=f"b_{nm}")
                nc.sync.dma_start(t[:], d[:].rearrange("t p o -> p t o"))
                b_sb[nm] = t
            bl_sb = constp.tile([1, 1], FP32)
            nc.sync.dma_start(bl_sb[:], bl_d[:])
            beta_sb = constp.tile([128, 1], FP32)
            nc.sync.dma_start(beta_sb[:], beta_d[:])

            out_sb = constp.tile([1, EC], FP32)

            # MLP layer, feature-major fp32r (fp22 reads, 1 cyc/row), 512 edges
            def lin_h(src, wname, bname, relu, dst):
                w, bias = w_sb[wname], b_sb[bname]
                for t in range(2):
                    pm = pmp.tile([128, EB], FP32, tag="pm")
                    for k in range(2):
                        nc.tensor.matmul(
                            pm[:], w[:, k, t * 128:(t + 1) * 128],
                            src[:, k, :], start=(k == 0), stop=(k == 1))
                    dsl = dst[:, t, :]
                    if t % 2 == 0:
                        nc.scalar.activation(
                            dsl, pm[:], AF.Relu if relu else AF.Identity,
                            bias=bias[:, t, :])
                    elif relu:
                        nc.vector.tensor_scalar(
                            dsl, pm[:], bias[:, t, :], 0.0,
                            mybir.AluOpType.add, mybir.AluOpType.max)
                    else:
                        nc.vector.tensor_scalar_add(dsl, pm[:], bias[:, t, :])
                return dst

            def mlp_block(b, xcn_sb, xiT, xjT):
                pT = actp.tile([128, 2, EB], FP32R, tag="act")
                nc.vector.tensor_mul(pT[:], xiT, xjT)
                u = lin_h(pT, "wat", "ba", True,
                          actp.tile([128, 2, EB], FP32R, tag="act", name=f"u{b}"))
                xijT = lin_h(u, "wbt", "bb", False,
                             actp.tile([128, 2, EB], FP32R, tag="act",
                                       name=f"xij{b}"))
                h = xcn_sb
                for li, (wn, bn, rl) in enumerate((
                        ("w1t", "b1", True), ("w2t", "b2", True),
                        ("w3t", "b3", False))):
                    h = lin_h(h, wn, bn, rl,
                              actp.tile([128, 2, EB], FP32R, tag="act",
                                        name=f"h{b}_{li}"))
                nc.vector.tensor_scalar_mul(h[:], h[:], beta_sb[:])
                nc.vector.tensor_add(h[:], h[:], xijT[:])
                po = pop.tile([1, EB], FP32, tag="po")
                for k in range(2):
                    nc.tensor.matmul(po[:], wlt_sb[:, k, :], h[:, k, :],
                                     start=(k == 0), stop=(k == 1))
                nc.scalar.activation(out_sb[:, b * EB:(b + 1) * EB],
                                     po[:], AF.Identity, bias=bl_sb[:])

            # ---- main loop: gather -> AND -> xcn^T matmul ------------
            for b in range(NB):
                px = [pxp.tile([128, EB], FP32, tag="px", name=f"px{b}_{fh}")
                      for fh in range(2)]
                xv = None
                for q in range(NSPLIT):
                    nch = QELEM[q] // 128          # byte-chunks in tile
                    ac2 = QC2[q]                   # adjacency word-chunks
                    ab = []
                    for s in range(2):
                        gsl = slice((2 * b + s) * EB // 16,
                                    (2 * b + s + 1) * EB // 16)
                        t = gathp.tile([128, nch, EB], FP8, tag=f"g{q}",
                                       bufs=3, name=f"a{b}{q}{s}")
                        nc.gpsimd.dma_gather(
                            t[:], adjx[:, QOFF[q]:QOFF[q] + QELEM[q]],
                            idxg_sb[:, gsl], EB, EB,
                            elem_size=QELEM[q], elem_step=ROWB, transpose=True)
                        ab.append(t)
                    if q == NSPLIT - 1:
                        # bf16 x rows ride in the last 4 byte-chunks:
                        # feature (fh*128+p), edge i at [p, 2*ac2+2*fh+i//256,
                        # i%256] of the bf16 view -> [128, 2, 512]
                        xv = [t[:].bitcast(BF16)[:, 2 * ac2:2 * ac2 + 4, :]
                              .rearrange("p (f s) w -> p f (s w)", f=2)
                              for t in ab]
                    # cn = a0 AND a1 (binary fp8: bitwise AND == product),
                    # in place into a0, contiguous u16 for DVE 2x mode;
                    # split for AND->matmul overlap, adjacency chunks only
                    v0 = ab[0][:].bitcast(I16)
                    v1 = ab[1][:].bitcast(I16)
                    for hh in range(2):
                        csl = slice(hh * ac2, (hh + 1) * ac2)
                        nc.vector.tensor_tensor(
                            v0[:, csl, :], v0[:, csl, :], v1[:, csl, :],
                            mybir.AluOpType.bitwise_and)
                    # cn byte at (c2, par, i): flat = c2*1024 + 2i + par
                    va = (ab[0][:, 0:2 * ac2, :]
                          .rearrange("p cb j -> p (cb j)")
                          .rearrange("p (c i two) -> p c two i",
                                     c=ac2, two=2))
                    for w2 in range(ac2 // 2):
                        gc = GBASE[q] + 2 * w2
                        for par in range(2):
                            for fh in range(2):
                                if DOUBLE_ROW:
                                    nc.tensor.matmul(
                                        px[fh][:],
                                        x8t_sb[:, gc:gc + 2, par, fh, :],
                                        va[:, 2 * w2:2 * w2 + 2, par, :],
                                        start=(q == 0 and w2 == 0 and par == 0),
                                        stop=(q == NSPLIT - 1
                                              and w2 == ac2 // 2 - 1
                                              and par == 1),
                                        perf_mode=mybir.MatmulPerfMode.DoubleRow)
                                else:
                                    for t2 in range(2):
                                        nc.tensor.matmul(
                                            px[fh][:],
                                            x8t_sb[:, gc + t2, par, fh, :],
                                            va[:, 2 * w2 + t2, par, :],
                                            start=(q == 0 and w2 == 0
                                                   and par == 0 and t2 == 0),
                                            stop=(q == NSPLIT - 1
                                                  and w2 == ac2 // 2 - 1
                                                  and par == 1 and t2 == 1))
                xcn_sb = actp.tile([128, 2, EB], FP32R, tag="act",
                                   name=f"xcn{b}")
                for fh in range(2):
                    nc.scalar.activation(xcn_sb[:, fh, :], px[fh][:], AF.Copy)
                mlp_block(b, xcn_sb, xv[0], xv[1])

            nc.sync.dma_start(out_d[:], out_sb[:])

    nc.compile()
    return nc


def _wrap_idx(ids, num):
    """Pack indices for dma_gather: [128, num//16] int16, idx i at
    [i % 16, i // 16], replicated over the 8 groups of 16 partitions."""
    a = np.asarray(ids).astype(np.int16)
    w = a.reshape(num // 16, 16).T.copy()
    return np.ascontiguousarray(np.tile(w, (8, 1)))


def prepare_inputs(x, adj, edge, W1, b1, W2, b2, W3, b3, Wa, ba, Wb, bb,
                   Wl, bl, beta):
    x = np.asarray(x, np.float32)
    adj = np.asarray(adj, np.float32)
    edge = np.asarray(edge)

    # extended rows: [adj fp8 (10240B) | x bf16 (512B)]
    adjx8 = np.zeros((N, ROWB), np.uint8)
    adjx8[:, :N] = adj.astype(FP8_NP).view(np.uint8)
    adjx8[:, NPAD:] = np.ascontiguousarray(
        x.astype(BF16_NP)).view(np.uint8).reshape(N, 2 * D)
    adjx = adjx8.view(FP8_NP)

    # fp8 x table, permuted to the gather interleave:
    # x8t[p, gc, par, fh, f] = x8[gc*256 + 2p + par, fh*128 + f]
    x8 = np.zeros((NPAD, D), FP8_NP)
    x8[:N] = x.astype(FP8_NP)
    x8t = np.ascontiguousarray(
        x8.reshape(NGC, 128, 2, 2, 128)
        .transpose(1, 0, 2, 3, 4).reshape(128, -1))

    common = dict(
        adjx=adjx, x8t=x8t,
        wat=np.ascontiguousarray(np.asarray(Wa, np.float32).T),
        wbt=np.ascontiguousarray(np.asarray(Wb, np.float32).T),
        w1t=np.ascontiguousarray(np.asarray(W1, np.float32).T),
        w2t=np.ascontiguousarray(np.asarray(W2, np.float32).T),
        w3t=np.ascontiguousarray(np.asarray(W3, np.float32).T),
        wlt=np.ascontiguousarray(np.asarray(Wl, np.float32).T),
        ba=np.asarray(ba, np.float32).reshape(2, 128, 1),
        bb=np.asarray(bb, np.float32).reshape(2, 128, 1),
        b1=np.asarray(b1, np.float32).reshape(2, 128, 1),
        b2=np.asarray(b2, np.float32).reshape(2, 128, 1),
        b3=np.asarray(b3, np.float32).reshape(2, 128, 1),
        bl=np.asarray(bl, np.float32).reshape(1, 1),
        beta=np.full((128, 1), np.asarray(beta, np.float32).reshape(-1)[0],
                     np.float32),
    )
    in_maps = []
    for c in range(N_CORES):
        m = dict(common)
        gi = []
        for b in range(NB):
            sl = slice(c * EC + b * EB, c * EC + (b + 1) * EB)
            for s in range(2):
                gi.append(_wrap_idx(edge[sl, s], EB))
        m["idxg"] = np.ascontiguousarray(np.hstack(gi))
        in_maps.append(m)
    return in_maps


_CACHE = {}


def _get_program():
    if "nc" not in _CACHE:
        _CACHE["nc"] = build_program()
    return _CACHE["nc"]


def run(in_maps, **kw):
    nc = _get_program()
    return run_bass_kernel_spmd(nc, in_maps, list(range(N_CORES)), **kw)


def kernel(**inputs):
    in_maps = prepare_inputs(**inputs)
    res = run(in_maps)
    out = np.concatenate([res.results[c]["out"][0] for c in range(N_CORES)])
    return out.reshape(E, 1).astype(np.float32)
